# revision 6
# baseline (speedup 1.0000x reference)
"""Grouped-experts MLP (MoE) kernel for Trainium2, expert-parallel over 8 cores.

Problem: x[B=2, E=8, N=1024, D=1024]; per expert e:
    out[:, e] = GELU(x[:, e] @ w1[e] + b1[e]) @ w2[e] + b2[e]
with w1[e]: [D=1024, H=4096], w2[e]: [H=4096, D=1024].

Sharding: expert axis across the 8 NeuronCores (core e owns expert e).
The host performs the "all-to-all": it hands core e the slab x[:, e]
plus expert e's weights and reassembles the full output afterward.

Per-core kernel (T = B*N = 2048 tokens), all matmul operands bf16 with
fp32 PSUM accumulation. The PE floor is 2048 matmuls x 512 cols
(~443us at 2.4GHz); everything else is startup/tail/bubble control:

  - All inputs are host-packed into SBUF-image DRAM tensors [128, N]
    laid out in exact consumption order, so every DMA is one large
    contiguous 2D slice (few DMA instructions, each ~0.6us to issue).
  - The first-wave DMAs (x quarter 0 + w1 column chunk 0, 2MB) are
    issued from scalar/vector/gpsimd, whose framework preambles finish
    ~1.3us before sync's; big follow-up loads stream on sync.
  - 4 warm-up matmuls bridge the HAM clock ramp; real matmuls start
    as soon as the first 256KB x/w1 slices land (~9us).
  - Layer 1 computes hT[h][128, 512] per H-tile (PSUM accum over 8
    D-tiles), GELU + b1 fused into the PSUM->SBUF eviction on the
    scalar engine, writing bf16. ps1 pool has 5 PSUM banks so the
    687ns GELU eviction never blocks a bank WAR (was a 53ns stall
    every few groups with 4 banks).
  - Layer 2 accumulates 32 H-tiles of a [128 tok, 512 dcol] tile in
    PSUM, adds b2 on the DVE, DMAs out. The very last group's
    eviction is split into 4x128-col slices DMAed from 4 different
    engines so the tail is ~64KB transfers instead of one 256KB.
"""

import numpy as np
import ml_dtypes

import concourse.bacc as bacc
import concourse.mybir as mybir
import concourse.tile as tile
from concourse.bass_utils import run_bass_kernel_spmd

B, E, N, D, H = 2, 8, 1024, 1024, 4096
T = B * N          # tokens per expert
P = 128
N_CORES = 8

TQ = 512           # tokens per quarter
NQ = T // TQ       # 4
KD = D // P        # 8 k-tiles over D (layer-1 contraction)
KH = H // P        # 32 k-tiles over H (layer-2 contraction)
HC = 8             # w1 column chunks of 512
HS = 4             # h-subtiles per w1 chunk
DC = D // 512      # 2 output column chunks
NTS = TQ // P      # 4 token subtiles per quarter

F32 = mybir.dt.float32
BF16 = mybir.dt.bfloat16
GELU = mybir.ActivationFunctionType.Gelu
BF16_NP = ml_dtypes.bfloat16


def build_nc():
    nc = bacc.Bacc("TRN2", target_bir_lowering=False, debug=False)

    # Host-packed SBUF images: [128, cols]; column layout is the exact
    # SBUF layout, so each DMA is a contiguous 2D slice.
    #   xin:  [q][k][512]   (q*4096 + k*512)
    #   w1in: [hc][k][512]  (hc*4096 + k*512)
    #   w2in: [dc][k][512]  (dc*16384 + k*512)
    xin = nc.dram_tensor("xin", [P, NQ * KD * TQ], BF16,
                         kind="ExternalInput")          # [128, 16384]
    w1in = nc.dram_tensor("w1in", [P, D * H // P], BF16,
                          kind="ExternalInput")         # [128, 32768]
    w2in = nc.dram_tensor("w2in", [P, H * D // P], BF16,
                          kind="ExternalInput")         # [128, 32768]
    b1 = nc.dram_tensor("b1", [P, KH], F32, kind="ExternalInput")
    b2 = nc.dram_tensor("b2", [P, D], F32, kind="ExternalInput")
    out = nc.dram_tensor("out", [T, D], F32, kind="ExternalOutput")

    with tile.TileContext(nc) as tc:
        with (
            tc.tile_pool(name="const", bufs=1) as constp,
            tc.tile_pool(name="xp", bufs=2) as xp,
            tc.tile_pool(name="w1p", bufs=1) as w1p,
            tc.tile_pool(name="w2p", bufs=1) as w2p,
            tc.tile_pool(name="hTp", bufs=1) as hTp,
            tc.tile_pool(name="stp", bufs=4) as stp,
            tc.tile_pool(name="ps1p", bufs=5, space="PSUM") as ps1p,
            tc.tile_pool(name="ps2p", bufs=3, space="PSUM") as ps2p,
        ):
            # ---- startup: first wave of DMAs spread across engines ----
            # x quarter tiles: 4 per quarter, [128, 1024] (k-pair each)
            def alloc_xq():
                return [xp.tile([P, 1024], BF16, name=f"x_{j}", tag=f"x_{j}")
                        for j in range(4)]

            def load_xq(eng_list, tiles, q):
                for j, t in enumerate(tiles):
                    eng_list[j % len(eng_list)].dma_start(
                        t[:], xin[:, q * 4096 + j * 1024:
                                  q * 4096 + (j + 1) * 1024])

            b1sb = constp.tile([P, KH], F32, name="b1sb")
            warm_src = constp.tile([P, 512], BF16, name="warm_src")

            xq_tiles = [None] * NQ
            xq_tiles[0] = alloc_xq()
            # w1 chunk 0 as 4 tiles [128,1024], chunk 1 as 2x[128,2048],
            # chunks 2..7 as one [128,4096] each
            w1c0 = [w1p.tile([P, 1024], BF16, name=f"w1c0_{j}")
                    for j in range(4)]
            w1c1 = [w1p.tile([P, 2048], BF16, name=f"w1c1_{j}")
                    for j in range(2)]
            w1c = {hc: w1p.tile([P, 4096], BF16, name=f"w1c_{hc}")
                   for hc in range(2, HC)}

            # scalar: b1 (needed by first GELU) then first x/w1 slices
            # (only sync/scalar/gpsimd can issue DMAs)
            nc.scalar.dma_start(b1sb[:], b1[:])
            nc.scalar.dma_start(xq_tiles[0][0][:], xin[:, 0:1024])
            nc.scalar.dma_start(w1c0[0][:], w1in[:, 0:1024])
            nc.scalar.dma_start(xq_tiles[0][1][:], xin[:, 1024:2048])
            nc.scalar.dma_start(w1c0[1][:], w1in[:, 1024:2048])
            # gpsimd: warm-up source memset first, then DMAs
            nc.gpsimd.memset(warm_src[:], 0.0)
            nc.gpsimd.dma_start(xq_tiles[0][2][:], xin[:, 2048:3072])
            nc.gpsimd.dma_start(w1c0[2][:], w1in[:, 2048:3072])
            # sync: the rest, in consumption order
            nc.sync.dma_start(xq_tiles[0][3][:], xin[:, 3072:4096])
            nc.sync.dma_start(w1c0[3][:], w1in[:, 3072:4096])
            for j in range(2):
                nc.sync.dma_start(
                    w1c1[j][:], w1in[:, 4096 + j * 2048:4096 + (j + 1) * 2048])
            for hc in range(2, HC):
                nc.sync.dma_start(
                    w1c[hc][:], w1in[:, hc * 4096:(hc + 1) * 4096])
            xq_tiles[1] = alloc_xq()
            load_xq([nc.sync], xq_tiles[1], 1)
            # w2: 2 dc-halves, 4 tiles [128,4096] each (k-octet per tile)
            w2t = {}
            for dc in range(DC):
                for i in range(4):
                    t = w2p.tile([P, 4096], BF16, name=f"w2_{dc}_{i}")
                    nc.sync.dma_start(
                        t[:], w2in[:, dc * 16384 + i * 4096:
                                   dc * 16384 + (i + 1) * 4096])
                    w2t[(dc, i)] = t
            b2sb = constp.tile([P, D], F32, name="b2sb")
            nc.sync.dma_start(b2sb[:], b2[:])

            # ---- HAM pre-warm: a few dependency-free matmuls keep the
            # PE busy from ~7us so the activity monitor un-throttles the
            # clock before real data lands (~9us). Garbage, never read.
            warm_ps = ps2p.tile([P, 512], F32, name="warm_ps", tag="ps2")
            for i in range(4):
                nc.tensor.matmul(
                    warm_ps[:], warm_src[:, 0:P], warm_src[:],
                    start=(i == 0), stop=(i == 3))

            def w1_ap(k, hc, hs):
                if hc == 0:
                    return w1c0[k // 2][:, (k % 2) * 512 + hs * P:
                                       (k % 2) * 512 + (hs + 1) * P]
                if hc == 1:
                    return w1c1[k // 4][:, (k % 4) * 512 + hs * P:
                                       (k % 4) * 512 + (hs + 1) * P]
                return w1c[hc][:, k * 512 + hs * P:k * 512 + (hs + 1) * P]

            def x_ap(xq, k):
                return xq[k // 2][:, (k % 2) * 512:(k % 2) * 512 + 512]

            def w2_ap(dc, k):
                return w2t[(dc, k // 8)][:, (k % 8) * 512:(k % 8) * 512 + 512]

            for q in range(NQ):
                xq = xq_tiles[q]

                # layer 1: hT[h] = GELU(w1[:, h-tile].T @ xq + b1[h-tile])
                hTt = []
                for h in range(KH):
                    hc, hs = h // HS, h % HS
                    ps = ps1p.tile([P, TQ], F32, name="ps1", tag="ps1")
                    for k in range(KD):
                        nc.tensor.matmul(
                            ps[:], w1_ap(k, hc, hs), x_ap(xq, k),
                            start=(k == 0), stop=(k == KD - 1))
                    ht = hTp.tile([P, TQ], BF16, name=f"hT_{h}", tag=f"hT_{h}")
                    nc.scalar.activation(
                        ht[:], ps[:], GELU, bias=b1sb[:, h:h + 1])
                    hTt.append(ht)

                # prefetch x for quarter q+2 (q/q+1 tiles both live;
                # pool bufs=2 recycles q's buffers once layer 1 is done)
                if q + 2 < NQ:
                    xq_tiles[q + 2] = alloc_xq()
                    load_xq([nc.sync], xq_tiles[q + 2], q + 2)

                # layer 2: out tile [128 tok, 512 dcol] accumulates all 32
                # H-tiles in PSUM, then +b2 on the DVE and straight to DRAM
                for dc in range(DC):
                    sl = slice(dc * 512, (dc + 1) * 512)
                    for ts in range(NTS):
                        t0 = q * TQ + ts * P
                        ps = ps2p.tile([P, 512], F32, name="ps2", tag="ps2")
                        for k in range(KH):
                            nc.tensor.matmul(
                                ps[:], hTt[k][:, ts * P:(ts + 1) * P],
                                w2_ap(dc, k),
                                start=(k == 0), stop=(k == KH - 1))
                        last = (q == NQ - 1 and dc == DC - 1 and ts == NTS - 1)
                        if not last:
                            st = stp.tile([P, 512], F32, name="st", tag="st")
                            nc.vector.tensor_add(st[:], b2sb[:, sl], ps[:])
                            nc.sync.dma_start(out[t0:t0 + P, sl], st[:])
                        else:
                            # tail: 4x128-col slices, each evicted on the
                            # DVE and DMAed from its own engine so the
                            # final transfer is 64KB, not 256KB
                            engs = [nc.sync, nc.scalar, nc.gpsimd, nc.sync]
                            for cc in range(4):
                                c0 = dc * 512 + cc * P
                                stf = constp.tile([P, P], F32,
                                                  name=f"stf_{cc}")
                                nc.vector.tensor_add(
                                    stf[:], b2sb[:, c0:c0 + P],
                                    ps[:, cc * P:(cc + 1) * P])
                                engs[cc].dma_start(
                                    out[t0:t0 + P, c0:c0 + P], stf[:])

    nc.compile()
    return nc


def make_in_map(x_e, w1_e, b1_e, w2_e, b2_e):
    """Per-core input map: pack one expert's slabs into SBUF images."""
    xT = x_e.reshape(T, D).T                      # [D, T]
    xin = np.ascontiguousarray(
        xT.reshape(KD, P, NQ, TQ).transpose(1, 2, 0, 3).reshape(P, -1)
    ).astype(BF16_NP)                             # [128, q*4096 + k*512]
    w1in = np.ascontiguousarray(
        w1_e.reshape(KD, P, HC, 512).transpose(1, 2, 0, 3).reshape(P, -1)
    ).astype(BF16_NP)                             # [128, hc*4096 + k*512]
    w2in = np.ascontiguousarray(
        w2_e.reshape(KH, P, DC, 512).transpose(1, 2, 0, 3).reshape(P, -1)
    ).astype(BF16_NP)                             # [128, dc*16384 + k*512]
    return {
        "xin": xin,
        "w1in": w1in,
        "w2in": w2in,
        "b1": np.ascontiguousarray(b1_e.reshape(KH, P).T),
        "b2": np.ascontiguousarray(
            np.broadcast_to(b2_e.reshape(1, D), (P, D))),
    }


_NC_CACHE = None


def _get_nc():
    global _NC_CACHE
    if _NC_CACHE is None:
        _NC_CACHE = build_nc()
    return _NC_CACHE


def kernel(x, w1, b1, w2, b2, trace=False):
    x = np.asarray(x, dtype=np.float32)
    w1 = np.asarray(w1, dtype=np.float32)
    b1 = np.asarray(b1, dtype=np.float32)
    w2 = np.asarray(w2, dtype=np.float32)
    b2 = np.asarray(b2, dtype=np.float32)

    nc = _get_nc()
    in_maps = [
        make_in_map(x[:, e], w1[e], b1[e], w2[e], b2[e]) for e in range(N_CORES)
    ]
    res = run_bass_kernel_spmd(
        nc, in_maps, core_ids=list(range(N_CORES)), trace=trace)
    out = np.empty((B, E, N, D), np.float32)
    for e in range(N_CORES):
        out[:, e] = res.results[e]["out"].reshape(B, N, D)
    if trace:
        return out, res
    return out


# revision 7
# speedup vs baseline: 1.0179x; 1.0179x over previous
"""Grouped-experts MLP (MoE) kernel for Trainium2, expert-parallel over 8 cores.

Problem: x[B=2, E=8, N=1024, D=1024]; per expert e:
    out[:, e] = GELU(x[:, e] @ w1[e] + b1[e]) @ w2[e] + b2[e]
with w1[e]: [D=1024, H=4096], w2[e]: [H=4096, D=1024].

Sharding: expert axis across the 8 NeuronCores (core e owns expert e).
The host performs the "all-to-all": it hands core e the slab x[:, e]
plus expert e's weights and reassembles the full output afterward.

Per-core kernel (T = B*N = 2048 tokens), all matmul operands bf16 with
fp32 PSUM accumulation. The PE floor is 2048 matmuls x 512 cols
(~443us at 2.4GHz); everything else is startup/tail/bubble control:

  - All inputs are host-packed into SBUF-image DRAM tensors [128, N]
    so every DMA is one large contiguous 2D slice. Large rows (2KB+)
    nearly double aggregate DMA bandwidth vs 1KB rows (~380 vs ~200
    GB/s sustained).
  - DMA completion tracks issue order (packets are FIFO behind earlier
    DMAs), so DMAs are issued in exact consumption order, interleaved
    across the three DMA-capable engines (sync/scalar/gpsimd; scalar
    and gpsimd preambles end ~1.3us before sync's first usable slot).
    w1 is packed h-major so each 256KB DMA feeds exactly one layer-1
    h-group: startup needs only x-q0 (1MB) + one w1 h-tile before the
    PE can run, and each further h-tile (0.7us transfer) covers 1.73us
    of PE work.
  - 4 warm-up matmuls bridge the HAM clock ramp from the framework
    preamble end (~7.2us) until the first x/w1 slices land.
  - Layer 1 computes hT[h][128, 512] per h-group (PSUM accum over 8
    D-tiles), GELU + b1 fused into the PSUM->SBUF eviction on the
    scalar engine, writing bf16.
  - Layer 2 accumulates 32 H-tiles of a [128 tok, 512 dcol] tile in
    PSUM, adds b2 on the DVE, DMAs out. The very last group's
    eviction is split into 4x128-col slices DMAed alternately from
    sync and scalar (gpsimd's SWDGE drain is ~2.6us — never on the
    tail) so the final transfer is 64KB, not 256KB.
"""

import numpy as np
import ml_dtypes

import concourse.bacc as bacc
import concourse.mybir as mybir
import concourse.tile as tile
from concourse.bass_utils import run_bass_kernel_spmd

B, E, N, D, H = 2, 8, 1024, 1024, 4096
T = B * N          # tokens per expert
P = 128
N_CORES = 8

TQ = 512           # tokens per quarter
NQ = T // TQ       # 4
KD = D // P        # 8 k-tiles over D (layer-1 contraction)
KH = H // P        # 32 k-tiles over H (layer-2 contraction)
DC = D // 512      # 2 output column chunks
NTS = TQ // P      # 4 token subtiles per quarter

F32 = mybir.dt.float32
BF16 = mybir.dt.bfloat16
GELU = mybir.ActivationFunctionType.Gelu
BF16_NP = ml_dtypes.bfloat16


def build_nc():
    nc = bacc.Bacc("TRN2", target_bir_lowering=False, debug=False)

    # Host-packed SBUF images: [128, cols]; column layout is the exact
    # SBUF layout, so each DMA is a contiguous 2D slice.
    #   xin:  [q][k][512]  (q*4096 + k*512)
    #   w1in: [h][k][128]  (h*1024 + k*128)   h-major: DMA unit = h-group
    #   w2in: [dc][k][512] (dc*16384 + k*512)
    xin = nc.dram_tensor("xin", [P, NQ * KD * TQ], BF16,
                         kind="ExternalInput")          # [128, 16384]
    w1in = nc.dram_tensor("w1in", [P, D * H // P], BF16,
                          kind="ExternalInput")         # [128, 32768]
    w2in = nc.dram_tensor("w2in", [P, H * D // P], BF16,
                          kind="ExternalInput")         # [128, 32768]
    b1 = nc.dram_tensor("b1", [P, KH], F32, kind="ExternalInput")
    b2 = nc.dram_tensor("b2", [P, D], F32, kind="ExternalInput")
    out = nc.dram_tensor("out", [T, D], F32, kind="ExternalOutput")

    with tile.TileContext(nc) as tc:
        with (
            tc.tile_pool(name="const", bufs=1) as constp,
            tc.tile_pool(name="xp", bufs=2) as xp,
            tc.tile_pool(name="w1p", bufs=1) as w1p,
            tc.tile_pool(name="w2p", bufs=1) as w2p,
            tc.tile_pool(name="hTp", bufs=1) as hTp,
            tc.tile_pool(name="stp", bufs=4) as stp,
            tc.tile_pool(name="ps1p", bufs=5, space="PSUM") as ps1p,
            tc.tile_pool(name="ps2p", bufs=3, space="PSUM") as ps2p,
        ):
            # x quarter tiles: 4 per quarter, [128, 1024] (k-pair each)
            def alloc_xq():
                return [xp.tile([P, 1024], BF16, name=f"x_{j}", tag=f"x_{j}")
                        for j in range(4)]

            def load_xq(eng, tiles, q):
                for j, t in enumerate(tiles):
                    eng.dma_start(
                        t[:], xin[:, q * 4096 + j * 1024:
                                  q * 4096 + (j + 1) * 1024])

            b1sb = constp.tile([P, KH], F32, name="b1sb")
            warm_src = constp.tile([P, 512], BF16, name="warm_src")

            xq_tiles = [None] * NQ
            xq_tiles[0] = alloc_xq()
            w1h = [w1p.tile([P, KD * P], BF16, name=f"w1h_{h}")
                   for h in range(KH)]

            # ---- startup DMAs, interleaved in consumption order ----
            # scalar / gpsimd handle x-q0 (needed first and throughout
            # every h-group); sync streams the w1 h-tiles in h order.
            nc.scalar.dma_start(xq_tiles[0][0][:], xin[:, 0:1024])
            nc.gpsimd.memset(warm_src[:], 0.0)
            nc.gpsimd.dma_start(xq_tiles[0][1][:], xin[:, 1024:2048])
            nc.sync.dma_start(w1h[0][:], w1in[:, 0:1024])
            nc.scalar.dma_start(xq_tiles[0][2][:], xin[:, 2048:3072])
            nc.gpsimd.dma_start(xq_tiles[0][3][:], xin[:, 3072:4096])
            nc.sync.dma_start(w1h[1][:], w1in[:, 1024:2048])
            nc.scalar.dma_start(b1sb[:], b1[:])
            for h in range(2, KH):
                nc.sync.dma_start(w1h[h][:], w1in[:, h * 1024:(h + 1) * 1024])
            xq_tiles[1] = alloc_xq()
            load_xq(nc.sync, xq_tiles[1], 1)
            # w2: 2 dc-halves, 4 tiles [128,4096] each (k-octet per tile)
            w2t = {}
            for dc in range(DC):
                for i in range(4):
                    t = w2p.tile([P, 4096], BF16, name=f"w2_{dc}_{i}")
                    nc.sync.dma_start(
                        t[:], w2in[:, dc * 16384 + i * 4096:
                                   dc * 16384 + (i + 1) * 4096])
                    w2t[(dc, i)] = t
            b2sb = constp.tile([P, D], F32, name="b2sb")
            nc.sync.dma_start(b2sb[:], b2[:])

            # ---- HAM pre-warm: a few dependency-free matmuls keep the
            # PE busy from ~7.2us so the activity monitor un-throttles
            # the clock before real data lands. Garbage, never read.
            warm_ps = ps2p.tile([P, 512], F32, name="warm_ps", tag="ps2")
            for i in range(4):
                nc.tensor.matmul(
                    warm_ps[:], warm_src[:, 0:P], warm_src[:],
                    start=(i == 0), stop=(i == 3))

            def x_ap(xq, k):
                return xq[k // 2][:, (k % 2) * 512:(k % 2) * 512 + 512]

            def w2_ap(dc, k):
                return w2t[(dc, k // 8)][:, (k % 8) * 512:(k % 8) * 512 + 512]

            for q in range(NQ):
                xq = xq_tiles[q]

                # layer 1: hT[h] = GELU(w1[h].T @ xq + b1[h])
                hTt = []
                for h in range(KH):
                    ps = ps1p.tile([P, TQ], F32, name="ps1", tag="ps1")
                    for k in range(KD):
                        nc.tensor.matmul(
                            ps[:], w1h[h][:, k * P:(k + 1) * P], x_ap(xq, k),
                            start=(k == 0), stop=(k == KD - 1))
                    ht = hTp.tile([P, TQ], BF16, name=f"hT_{h}", tag=f"hT_{h}")
                    nc.scalar.activation(
                        ht[:], ps[:], GELU, bias=b1sb[:, h:h + 1])
                    hTt.append(ht)

                # prefetch x for quarter q+2 (q/q+1 tiles both live;
                # pool bufs=2 recycles q's buffers once layer 1 is done)
                if q + 2 < NQ:
                    xq_tiles[q + 2] = alloc_xq()
                    load_xq(nc.sync, xq_tiles[q + 2], q + 2)

                # layer 2: out tile [128 tok, 512 dcol] accumulates all 32
                # H-tiles in PSUM, then +b2 on the DVE and straight to DRAM
                for dc in range(DC):
                    sl = slice(dc * 512, (dc + 1) * 512)
                    for ts in range(NTS):
                        t0 = q * TQ + ts * P
                        ps = ps2p.tile([P, 512], F32, name="ps2", tag="ps2")
                        for k in range(KH):
                            nc.tensor.matmul(
                                ps[:], hTt[k][:, ts * P:(ts + 1) * P],
                                w2_ap(dc, k),
                                start=(k == 0), stop=(k == KH - 1))
                        last = (q == NQ - 1 and dc == DC - 1 and ts == NTS - 1)
                        if not last:
                            st = stp.tile([P, 512], F32, name="st", tag="st")
                            nc.vector.tensor_add(st[:], b2sb[:, sl], ps[:])
                            nc.sync.dma_start(out[t0:t0 + P, sl], st[:])
                        else:
                            # tail: 4x128-col slices, each evicted on the
                            # DVE and DMAed from sync/scalar so the final
                            # transfer is 64KB, not 256KB
                            engs = [nc.sync, nc.scalar, nc.sync, nc.scalar]
                            for cc in range(4):
                                c0 = dc * 512 + cc * P
                                stf = constp.tile([P, P], F32,
                                                  name=f"stf_{cc}")
                                nc.vector.tensor_add(
                                    stf[:], b2sb[:, c0:c0 + P],
                                    ps[:, cc * P:(cc + 1) * P])
                                engs[cc].dma_start(
                                    out[t0:t0 + P, c0:c0 + P], stf[:])

    nc.compile()
    return nc


def make_in_map(x_e, w1_e, b1_e, w2_e, b2_e):
    """Per-core input map: pack one expert's slabs into SBUF images."""
    xT = x_e.reshape(T, D).T                      # [D, T]
    xin = np.ascontiguousarray(
        xT.reshape(KD, P, NQ, TQ).transpose(1, 2, 0, 3).reshape(P, -1)
    ).astype(BF16_NP)                             # [128, q*4096 + k*512]
    w1in = np.ascontiguousarray(
        w1_e.reshape(KD, P, KH, P).transpose(1, 2, 0, 3).reshape(P, -1)
    ).astype(BF16_NP)                             # [128, h*1024 + k*128]
    w2in = np.ascontiguousarray(
        w2_e.reshape(KH, P, DC, 512).transpose(1, 2, 0, 3).reshape(P, -1)
    ).astype(BF16_NP)                             # [128, dc*16384 + k*512]
    return {
        "xin": xin,
        "w1in": w1in,
        "w2in": w2in,
        "b1": np.ascontiguousarray(b1_e.reshape(KH, P).T),
        "b2": np.ascontiguousarray(
            np.broadcast_to(b2_e.reshape(1, D), (P, D))),
    }


_NC_CACHE = None


def _get_nc():
    global _NC_CACHE
    if _NC_CACHE is None:
        _NC_CACHE = build_nc()
    return _NC_CACHE


def kernel(x, w1, b1, w2, b2, trace=False):
    x = np.asarray(x, dtype=np.float32)
    w1 = np.asarray(w1, dtype=np.float32)
    b1 = np.asarray(b1, dtype=np.float32)
    w2 = np.asarray(w2, dtype=np.float32)
    b2 = np.asarray(b2, dtype=np.float32)

    nc = _get_nc()
    in_maps = [
        make_in_map(x[:, e], w1[e], b1[e], w2[e], b2[e]) for e in range(N_CORES)
    ]
    res = run_bass_kernel_spmd(
        nc, in_maps, core_ids=list(range(N_CORES)), trace=trace)
    out = np.empty((B, E, N, D), np.float32)
    for e in range(N_CORES):
        out[:, e] = res.results[e]["out"].reshape(B, N, D)
    if trace:
        return out, res
    return out


# revision 10
# speedup vs baseline: 1.0225x; 1.0046x over previous
"""Grouped-experts MLP (MoE) kernel for Trainium2, expert-parallel over 8 cores.

Problem: x[B=2, E=8, N=1024, D=1024]; per expert e:
    out[:, e] = GELU(x[:, e] @ w1[e] + b1[e]) @ w2[e] + b2[e]
with w1[e]: [D=1024, H=4096], w2[e]: [H=4096, D=1024].

Sharding: expert axis across the 8 NeuronCores (core e owns expert e).
The host performs the "all-to-all": it hands core e the slab x[:, e]
plus expert e's weights and reassembles the full output afterward.

Per-core kernel (T = B*N = 2048 tokens), all matmul operands bf16 with
fp32 PSUM accumulation. The PE floor is 2048 matmuls x 512 cols
(~443us at 2.4GHz); everything else is startup/tail/bubble control:

  - All inputs are host-packed into SBUF-image DRAM tensors [128, N]
    so every DMA is one large contiguous 2D slice. Large rows (2KB+)
    nearly double aggregate DMA bandwidth vs 1KB rows (~380 vs ~200
    GB/s sustained).
  - DMA completion tracks issue order (packets are FIFO behind earlier
    DMAs), so DMAs are issued in exact consumption order, interleaved
    across the three DMA-capable engines (sync/scalar/gpsimd; scalar
    and gpsimd preambles end ~1.3us before sync's first usable slot).
    w1 is packed h-major so each 256KB DMA feeds exactly one layer-1
    h-group: startup needs only x-q0 (1MB) + one w1 h-tile before the
    PE can run, and each further h-tile (0.7us transfer) covers 1.73us
    of PE work.
  - 4 warm-up matmuls bridge the HAM clock ramp from the framework
    preamble end (~7.2us) until the first x/w1 slices land.
  - Layer 1 computes hT[h][128, 512] per h-group (PSUM accum over 8
    D-tiles), GELU + b1 fused into the PSUM->SBUF eviction on the
    scalar engine, writing bf16.
  - Layer 2 accumulates 32 H-tiles of a [128 tok, 512 dcol] tile in
    PSUM, adds b2 on the DVE, DMAs out. The very last group's
    eviction is split into 4x128-col slices DMAed alternately from
    sync and scalar (gpsimd's SWDGE drain is ~2.6us — never on the
    tail) so the final transfer is 64KB, not 256KB.
"""

import numpy as np
import ml_dtypes

import concourse.bacc as bacc
import concourse.mybir as mybir
import concourse.tile as tile
from concourse.bass_utils import run_bass_kernel_spmd

B, E, N, D, H = 2, 8, 1024, 1024, 4096
T = B * N          # tokens per expert
P = 128
N_CORES = 8

TQ = 512           # tokens per quarter
NQ = T // TQ       # 4
KD = D // P        # 8 k-tiles over D (layer-1 contraction)
KH = H // P        # 32 k-tiles over H (layer-2 contraction)
DC = D // 512      # 2 output column chunks
NTS = TQ // P      # 4 token subtiles per quarter

F32 = mybir.dt.float32
BF16 = mybir.dt.bfloat16
GELU = mybir.ActivationFunctionType.Gelu
BF16_NP = ml_dtypes.bfloat16


def build_nc():
    nc = bacc.Bacc("TRN2", target_bir_lowering=False, debug=False)

    # Host-packed SBUF images: [128, cols]; column layout is the exact
    # SBUF layout, so each DMA is a contiguous 2D slice.
    #   xin:  [q][k][512]  (q*4096 + k*512)
    #   w1in: [h][k][128]  (h*1024 + k*128)   h-major: DMA unit = h-group
    #   w2in: [dc][k][512] (dc*16384 + k*512)
    xin = nc.dram_tensor("xin", [P, NQ * KD * TQ], BF16,
                         kind="ExternalInput")          # [128, 16384]
    w1in = nc.dram_tensor("w1in", [P, D * H // P], BF16,
                          kind="ExternalInput")         # [128, 32768]
    w2in = nc.dram_tensor("w2in", [P, H * D // P], BF16,
                          kind="ExternalInput")         # [128, 32768]
    b1 = nc.dram_tensor("b1", [P, KH], F32, kind="ExternalInput")
    b2 = nc.dram_tensor("b2", [P, D], F32, kind="ExternalInput")
    out = nc.dram_tensor("out", [T, D], F32, kind="ExternalOutput")

    with tile.TileContext(nc) as tc:
        with (
            tc.tile_pool(name="const", bufs=1) as constp,
            tc.tile_pool(name="xp", bufs=2) as xp,
            tc.tile_pool(name="w1p", bufs=1) as w1p,
            tc.tile_pool(name="w2p", bufs=1) as w2p,
            tc.tile_pool(name="hTp", bufs=1) as hTp,
            tc.tile_pool(name="stp", bufs=4) as stp,
            tc.tile_pool(name="ps1p", bufs=5, space="PSUM") as ps1p,
            tc.tile_pool(name="ps2p", bufs=3, space="PSUM") as ps2p,
        ):
            # x quarter tiles: 4 per quarter, [128, 1024] (k-pair each)
            def alloc_xq():
                return [xp.tile([P, 1024], BF16, name=f"x_{j}", tag=f"x_{j}")
                        for j in range(4)]

            def load_xq(eng, tiles, q):
                for j, t in enumerate(tiles):
                    eng.dma_start(
                        t[:], xin[:, q * 4096 + j * 1024:
                                  q * 4096 + (j + 1) * 1024])

            b1sb = constp.tile([P, KH], F32, name="b1sb")
            warm_src = constp.tile([P, 512], BF16, name="warm_src")

            xq_tiles = [None] * NQ
            xq_tiles[0] = alloc_xq()
            w1h = [w1p.tile([P, KD * P], BF16, name=f"w1h_{h}")
                   for h in range(KH)]

            # ---- startup DMAs, interleaved in consumption order ----
            # DMA completion tracks issue order, so the critical first
            # group's data (w1h0 + x-q0) takes the very first slot on
            # each DMA-capable engine; w1 h-tiles then stream on sync.
            nc.sync.dma_start(w1h[0][:], w1in[:, 0:1024])
            nc.scalar.dma_start(xq_tiles[0][0][:], xin[:, 0:1024])
            nc.gpsimd.memset(warm_src[:], 0.0)
            nc.gpsimd.dma_start(xq_tiles[0][1][:], xin[:, 1024:2048])
            nc.scalar.dma_start(xq_tiles[0][2][:], xin[:, 2048:3072])
            nc.gpsimd.dma_start(xq_tiles[0][3][:], xin[:, 3072:4096])
            nc.sync.dma_start(w1h[1][:], w1in[:, 1024:2048])
            nc.scalar.dma_start(b1sb[:], b1[:])
            for h in range(2, KH):
                nc.sync.dma_start(w1h[h][:], w1in[:, h * 1024:(h + 1) * 1024])
            xq_tiles[1] = alloc_xq()
            load_xq(nc.sync, xq_tiles[1], 1)
            # w2: 2 dc-halves, 4 tiles [128,4096] each (k-octet per tile)
            w2t = {}
            for dc in range(DC):
                for i in range(4):
                    t = w2p.tile([P, 4096], BF16, name=f"w2_{dc}_{i}")
                    nc.sync.dma_start(
                        t[:], w2in[:, dc * 16384 + i * 4096:
                                   dc * 16384 + (i + 1) * 4096])
                    w2t[(dc, i)] = t
            b2sb = constp.tile([P, D], F32, name="b2sb")
            nc.sync.dma_start(b2sb[:], b2[:])

            # ---- HAM pre-warm: a few dependency-free matmuls keep the
            # PE busy from ~7.2us so the activity monitor un-throttles
            # the clock before real data lands. Garbage, never read.
            warm_ps = ps2p.tile([P, 512], F32, name="warm_ps", tag="ps2")
            NWARM = 7
            for i in range(NWARM):
                nc.tensor.matmul(
                    warm_ps[:], warm_src[:, 0:P], warm_src[:],
                    start=(i == 0), stop=(i == NWARM - 1))

            def x_ap(xq, k):
                return xq[k // 2][:, (k % 2) * 512:(k % 2) * 512 + 512]

            def w2_ap(dc, k):
                return w2t[(dc, k // 8)][:, (k % 8) * 512:(k % 8) * 512 + 512]

            for q in range(NQ):
                xq = xq_tiles[q]

                # layer 1: hT[h] = GELU(w1[h].T @ xq + b1[h])
                hTt = []
                for h in range(KH):
                    ps = ps1p.tile([P, TQ], F32, name="ps1", tag="ps1")
                    for k in range(KD):
                        nc.tensor.matmul(
                            ps[:], w1h[h][:, k * P:(k + 1) * P], x_ap(xq, k),
                            start=(k == 0), stop=(k == KD - 1))
                    ht = hTp.tile([P, TQ], BF16, name=f"hT_{h}", tag=f"hT_{h}")
                    nc.scalar.activation(
                        ht[:], ps[:], GELU, bias=b1sb[:, h:h + 1])
                    hTt.append(ht)

                # prefetch x for quarter q+2 (q/q+1 tiles both live;
                # pool bufs=2 recycles q's buffers once layer 1 is done)
                if q + 2 < NQ:
                    xq_tiles[q + 2] = alloc_xq()
                    load_xq(nc.sync, xq_tiles[q + 2], q + 2)

                # layer 2: out tile [128 tok, 512 dcol] accumulates all 32
                # H-tiles in PSUM, then +b2 on the DVE and straight to DRAM
                for dc in range(DC):
                    sl = slice(dc * 512, (dc + 1) * 512)
                    for ts in range(NTS):
                        t0 = q * TQ + ts * P
                        ps = ps2p.tile([P, 512], F32, name="ps2", tag="ps2")
                        last = (q == NQ - 1 and dc == DC - 1 and ts == NTS - 1)
                        if not last:
                            for k in range(KH):
                                nc.tensor.matmul(
                                    ps[:], hTt[k][:, ts * P:(ts + 1) * P],
                                    w2_ap(dc, k),
                                    start=(k == 0), stop=(k == KH - 1))
                            st = stp.tile([P, 512], F32, name="st", tag="st")
                            nc.vector.tensor_add(st[:], b2sb[:, sl], ps[:])
                            nc.sync.dma_start(out[t0:t0 + P, sl], st[:])
                        else:
                            # tail: two 256-col accumulation sub-groups.
                            # Half A's eviction + 128KB DMA overlap half
                            # B's 32 matmuls; after the last matmul only
                            # a 256-col DVE add and two parallel 64KB
                            # DMAs (sync + scalar) remain.
                            for hf in range(2):
                                c0 = dc * 512 + hf * 256
                                for k in range(KH):
                                    w2a = w2t[(dc, k // 8)][
                                        :, (k % 8) * 512 + hf * 256:
                                        (k % 8) * 512 + hf * 256 + 256]
                                    nc.tensor.matmul(
                                        ps[:, hf * 256:hf * 256 + 256],
                                        hTt[k][:, ts * P:(ts + 1) * P],
                                        w2a,
                                        start=(k == 0), stop=(k == KH - 1))
                                stf = constp.tile([P, 256], F32,
                                                  name=f"stf_{hf}")
                                nc.vector.tensor_add(
                                    stf[:], b2sb[:, c0:c0 + 256],
                                    ps[:, hf * 256:hf * 256 + 256])
                                if hf == 0:
                                    nc.sync.dma_start(
                                        out[t0:t0 + P, c0:c0 + 256], stf[:])
                                else:
                                    nc.sync.dma_start(
                                        out[t0:t0 + P, c0:c0 + P],
                                        stf[:, 0:P])
                                    nc.scalar.dma_start(
                                        out[t0:t0 + P, c0 + P:c0 + 256],
                                        stf[:, P:256])

    nc.compile()
    return nc


def make_in_map(x_e, w1_e, b1_e, w2_e, b2_e):
    """Per-core input map: pack one expert's slabs into SBUF images."""
    xT = x_e.reshape(T, D).T                      # [D, T]
    xin = np.ascontiguousarray(
        xT.reshape(KD, P, NQ, TQ).transpose(1, 2, 0, 3).reshape(P, -1)
    ).astype(BF16_NP)                             # [128, q*4096 + k*512]
    w1in = np.ascontiguousarray(
        w1_e.reshape(KD, P, KH, P).transpose(1, 2, 0, 3).reshape(P, -1)
    ).astype(BF16_NP)                             # [128, h*1024 + k*128]
    w2in = np.ascontiguousarray(
        w2_e.reshape(KH, P, DC, 512).transpose(1, 2, 0, 3).reshape(P, -1)
    ).astype(BF16_NP)                             # [128, dc*16384 + k*512]
    return {
        "xin": xin,
        "w1in": w1in,
        "w2in": w2in,
        "b1": np.ascontiguousarray(b1_e.reshape(KH, P).T),
        "b2": np.ascontiguousarray(
            np.broadcast_to(b2_e.reshape(1, D), (P, D))),
    }


_NC_CACHE = None


def _get_nc():
    global _NC_CACHE
    if _NC_CACHE is None:
        _NC_CACHE = build_nc()
    return _NC_CACHE


def kernel(x, w1, b1, w2, b2, trace=False):
    x = np.asarray(x, dtype=np.float32)
    w1 = np.asarray(w1, dtype=np.float32)
    b1 = np.asarray(b1, dtype=np.float32)
    w2 = np.asarray(w2, dtype=np.float32)
    b2 = np.asarray(b2, dtype=np.float32)

    nc = _get_nc()
    in_maps = [
        make_in_map(x[:, e], w1[e], b1[e], w2[e], b2[e]) for e in range(N_CORES)
    ]
    res = run_bass_kernel_spmd(
        nc, in_maps, core_ids=list(range(N_CORES)), trace=trace)
    out = np.empty((B, E, N, D), np.float32)
    for e in range(N_CORES):
        out[:, e] = res.results[e]["out"].reshape(B, N, D)
    if trace:
        return out, res
    return out


# revision 14
# speedup vs baseline: 1.0277x; 1.0050x over previous
"""Grouped-experts MLP (MoE) kernel for Trainium2, expert-parallel over 8 cores.

Problem: x[B=2, E=8, N=1024, D=1024]; per expert e:
    out[:, e] = GELU(x[:, e] @ w1[e] + b1[e]) @ w2[e] + b2[e]
with w1[e]: [D=1024, H=4096], w2[e]: [H=4096, D=1024].

Sharding: expert axis across the 8 NeuronCores (core e owns expert e).
The host performs the "all-to-all": it hands core e the slab x[:, e]
plus expert e's weights and reassembles the full output afterward.

Per-core kernel (T = B*N = 2048 tokens), all matmul operands bf16 with
fp32 PSUM accumulation. The PE floor is 2048 matmuls x 512 cols
(~443us at 2.4GHz); everything else is startup/tail/bubble control:

  - All inputs are host-packed into SBUF-image DRAM tensors [128, N]
    so every DMA is one large contiguous 2D slice. Large rows (2KB+)
    nearly double aggregate DMA bandwidth vs 1KB rows (~380 vs ~200
    GB/s sustained).
  - DMA completion tracks issue order (packets are FIFO behind earlier
    DMAs), so DMAs are issued in exact consumption order, interleaved
    across the three DMA-capable engines (sync/scalar/gpsimd; scalar
    and gpsimd preambles end ~1.3us before sync's first usable slot).
    w1 is packed h-major so each 256KB DMA feeds exactly one layer-1
    h-group: startup needs only x-q0 (1MB) + one w1 h-tile before the
    PE can run, and each further h-tile (0.7us transfer) covers 1.73us
    of PE work.
  - 4 warm-up matmuls bridge the HAM clock ramp from the framework
    preamble end (~7.2us) until the first x/w1 slices land.
  - Layer 1 computes hT[h][128, 512] per h-group (PSUM accum over 8
    D-tiles), GELU + b1 fused into the PSUM->SBUF eviction on the
    scalar engine, writing bf16.
  - Layer 2 accumulates 32 H-tiles of a [128 tok, 512 dcol] tile in
    PSUM, adds b2 on the DVE, DMAs out. The very last group's
    eviction is split into 4x128-col slices DMAed alternately from
    sync and scalar (gpsimd's SWDGE drain is ~2.6us — never on the
    tail) so the final transfer is 64KB, not 256KB.
"""

import numpy as np
import ml_dtypes

import concourse.bacc as bacc
import concourse.mybir as mybir
import concourse.tile as tile
from concourse.bass_utils import run_bass_kernel_spmd

B, E, N, D, H = 2, 8, 1024, 1024, 4096
T = B * N          # tokens per expert
P = 128
N_CORES = 8

TQ = 512           # tokens per quarter
NQ = T // TQ       # 4
KD = D // P        # 8 k-tiles over D (layer-1 contraction)
KH = H // P        # 32 k-tiles over H (layer-2 contraction)
DC = D // 512      # 2 output column chunks
NTS = TQ // P      # 4 token subtiles per quarter

F32 = mybir.dt.float32
BF16 = mybir.dt.bfloat16
GELU = mybir.ActivationFunctionType.Gelu
BF16_NP = ml_dtypes.bfloat16


def build_nc():
    nc = bacc.Bacc("TRN2", target_bir_lowering=False, debug=False)

    # Host-packed SBUF images: [128, cols]; column layout is the exact
    # SBUF layout, so each DMA is a contiguous 2D slice.
    #   xin:  [q][k][512]  (q*4096 + k*512)
    #   w1in: [h][k][128]  (h*1024 + k*128)   h-major: DMA unit = h-group
    #   w2in: [dc][k][512] (dc*16384 + k*512)
    xin = nc.dram_tensor("xin", [P, NQ * KD * TQ], BF16,
                         kind="ExternalInput")          # [128, 16384]
    w1in = nc.dram_tensor("w1in", [P, D * H // P], BF16,
                          kind="ExternalInput")         # [128, 32768]
    w2in = nc.dram_tensor("w2in", [P, H * D // P], BF16,
                          kind="ExternalInput")         # [128, 32768]
    b1 = nc.dram_tensor("b1", [P, KH], F32, kind="ExternalInput")
    b2 = nc.dram_tensor("b2", [P, D], F32, kind="ExternalInput")
    out = nc.dram_tensor("out", [T, D], F32, kind="ExternalOutput")

    with tile.TileContext(nc) as tc:
        with (
            tc.tile_pool(name="const", bufs=1) as constp,
            tc.tile_pool(name="xp", bufs=2) as xp,
            tc.tile_pool(name="w1p", bufs=1) as w1p,
            tc.tile_pool(name="w2p", bufs=1) as w2p,
            tc.tile_pool(name="hTp", bufs=1) as hTp,
            tc.tile_pool(name="stp", bufs=4) as stp,
            tc.tile_pool(name="ps1p", bufs=5, space="PSUM") as ps1p,
            tc.tile_pool(name="ps2p", bufs=3, space="PSUM") as ps2p,
        ):
            # x quarter: one [128, 4096] tile per quarter (8KB DRAM rows
            # -> best DMA bandwidth, single FIFO train, 1 issue)
            def alloc_xq():
                return xp.tile([P, 4096], BF16, name="xq", tag="xq")

            def load_xq(eng, t, q):
                eng.dma_start(t[:], xin[:, q * 4096:(q + 1) * 4096])

            b1sb = constp.tile([P, KH], F32, name="b1sb")
            warm_src = constp.tile([P, 512], BF16, name="warm_src")

            xq_tiles = [None] * NQ
            xq_tiles[0] = alloc_xq()
            w1h = [w1p.tile([P, KD * P], BF16, name=f"w1h_{h}")
                   for h in range(KH)]

            # ---- startup DMAs, interleaved in consumption order ----
            # DMA completion tracks issue order, so the critical first
            # group's data (w1h0 + x-q0) takes the very first slot on
            # each DMA-capable engine; w1 h-tiles then stream on sync.
            nc.sync.dma_start(w1h[0][:], w1in[:, 0:1024])
            nc.scalar.dma_start(xq_tiles[0][:], xin[:, 0:4096])
            nc.gpsimd.memset(warm_src[:], 0.0)
            nc.gpsimd.dma_start(b1sb[:], b1[:])
            nc.gpsimd.dma_start(w1h[1][:], w1in[:, 1024:2048])
            for h in range(2, KH):
                nc.sync.dma_start(w1h[h][:], w1in[:, h * 1024:(h + 1) * 1024])
            xq_tiles[1] = alloc_xq()
            load_xq(nc.sync, xq_tiles[1], 1)
            # w2: 2 dc-halves, 4 tiles [128,4096] each (k-octet per tile)
            w2t = {}
            for dc in range(DC):
                for i in range(4):
                    t = w2p.tile([P, 4096], BF16, name=f"w2_{dc}_{i}")
                    nc.sync.dma_start(
                        t[:], w2in[:, dc * 16384 + i * 4096:
                                   dc * 16384 + (i + 1) * 4096])
                    w2t[(dc, i)] = t
            b2sb = constp.tile([P, D], F32, name="b2sb")
            nc.sync.dma_start(b2sb[:], b2[:])

            # ---- HAM pre-warm: a few dependency-free matmuls keep the
            # PE busy from ~7.2us so the activity monitor un-throttles
            # the clock before real data lands. Garbage, never read.
            warm_ps = ps2p.tile([P, 512], F32, name="warm_ps", tag="ps2")
            NWARM = 16
            for i in range(NWARM):
                nc.tensor.matmul(
                    warm_ps[:], warm_src[:, 0:P], warm_src[:],
                    start=(i == 0), stop=(i == NWARM - 1))

            def x_ap(xq, k):
                return xq[:, k * 512:(k + 1) * 512]

            def w2_ap(dc, k):
                return w2t[(dc, k // 8)][:, (k % 8) * 512:(k % 8) * 512 + 512]

            for q in range(NQ):
                xq = xq_tiles[q]

                # layer 1: hT[h] = GELU(w1[h].T @ xq + b1[h])
                hTt = []
                for h in range(KH):
                    ps = ps1p.tile([P, TQ], F32, name="ps1", tag="ps1")
                    for k in range(KD):
                        nc.tensor.matmul(
                            ps[:], w1h[h][:, k * P:(k + 1) * P], x_ap(xq, k),
                            start=(k == 0), stop=(k == KD - 1))
                    ht = hTp.tile([P, TQ], BF16, name=f"hT_{h}", tag=f"hT_{h}")
                    nc.scalar.activation(
                        ht[:], ps[:], GELU, bias=b1sb[:, h:h + 1])
                    hTt.append(ht)

                # prefetch x for quarter q+2 (q/q+1 tiles both live;
                # pool bufs=2 recycles q's buffers once layer 1 is done)
                if q + 2 < NQ:
                    xq_tiles[q + 2] = alloc_xq()
                    load_xq(nc.sync, xq_tiles[q + 2], q + 2)

                # layer 2: out tile [128 tok, 512 dcol] accumulates all 32
                # H-tiles in PSUM, then +b2 on the DVE and straight to DRAM
                for dc in range(DC):
                    sl = slice(dc * 512, (dc + 1) * 512)
                    for ts in range(NTS):
                        t0 = q * TQ + ts * P
                        ps = ps2p.tile([P, 512], F32, name="ps2", tag="ps2")
                        last = (q == NQ - 1 and dc == DC - 1 and ts == NTS - 1)
                        if not last:
                            for k in range(KH):
                                nc.tensor.matmul(
                                    ps[:], hTt[k][:, ts * P:(ts + 1) * P],
                                    w2_ap(dc, k),
                                    start=(k == 0), stop=(k == KH - 1))
                            st = stp.tile([P, 512], F32, name="st", tag="st")
                            nc.vector.tensor_add(st[:], b2sb[:, sl], ps[:])
                            nc.sync.dma_start(out[t0:t0 + P, sl], st[:])
                        else:
                            # tail: two 256-col accumulation sub-groups.
                            # Half A's eviction + 128KB DMA overlap half
                            # B's 32 matmuls; after the last matmul only
                            # a 256-col DVE add and two parallel 64KB
                            # DMAs (sync + scalar) remain.
                            for hf in range(2):
                                c0 = dc * 512 + hf * 256
                                for k in range(KH):
                                    w2a = w2t[(dc, k // 8)][
                                        :, (k % 8) * 512 + hf * 256:
                                        (k % 8) * 512 + hf * 256 + 256]
                                    nc.tensor.matmul(
                                        ps[:, hf * 256:hf * 256 + 256],
                                        hTt[k][:, ts * P:(ts + 1) * P],
                                        w2a,
                                        start=(k == 0), stop=(k == KH - 1))
                                stf = constp.tile([P, 256], F32,
                                                  name=f"stf_{hf}")
                                nc.vector.tensor_add(
                                    stf[:], b2sb[:, c0:c0 + 256],
                                    ps[:, hf * 256:hf * 256 + 256])
                                if hf == 0:
                                    nc.sync.dma_start(
                                        out[t0:t0 + P, c0:c0 + 256], stf[:])
                                else:
                                    nc.sync.dma_start(
                                        out[t0:t0 + P, c0:c0 + P],
                                        stf[:, 0:P])
                                    nc.scalar.dma_start(
                                        out[t0:t0 + P, c0 + P:c0 + 256],
                                        stf[:, P:256])

    nc.compile()
    return nc


def make_in_map(x_e, w1_e, b1_e, w2_e, b2_e):
    """Per-core input map: pack one expert's slabs into SBUF images."""
    xT = x_e.reshape(T, D).T                      # [D, T]
    xin = np.ascontiguousarray(
        xT.reshape(KD, P, NQ, TQ).transpose(1, 2, 0, 3).reshape(P, -1)
    ).astype(BF16_NP)                             # [128, q*4096 + k*512]
    w1in = np.ascontiguousarray(
        w1_e.reshape(KD, P, KH, P).transpose(1, 2, 0, 3).reshape(P, -1)
    ).astype(BF16_NP)                             # [128, h*1024 + k*128]
    w2in = np.ascontiguousarray(
        w2_e.reshape(KH, P, DC, 512).transpose(1, 2, 0, 3).reshape(P, -1)
    ).astype(BF16_NP)                             # [128, dc*16384 + k*512]
    return {
        "xin": xin,
        "w1in": w1in,
        "w2in": w2in,
        "b1": np.ascontiguousarray(b1_e.reshape(KH, P).T),
        "b2": np.ascontiguousarray(
            np.broadcast_to(b2_e.reshape(1, D), (P, D))),
    }


_NC_CACHE = None


def _get_nc():
    global _NC_CACHE
    if _NC_CACHE is None:
        _NC_CACHE = build_nc()
    return _NC_CACHE


def kernel(x, w1, b1, w2, b2, trace=False):
    x = np.asarray(x, dtype=np.float32)
    w1 = np.asarray(w1, dtype=np.float32)
    b1 = np.asarray(b1, dtype=np.float32)
    w2 = np.asarray(w2, dtype=np.float32)
    b2 = np.asarray(b2, dtype=np.float32)

    nc = _get_nc()
    in_maps = [
        make_in_map(x[:, e], w1[e], b1[e], w2[e], b2[e]) for e in range(N_CORES)
    ]
    res = run_bass_kernel_spmd(
        nc, in_maps, core_ids=list(range(N_CORES)), trace=trace)
    out = np.empty((B, E, N, D), np.float32)
    for e in range(N_CORES):
        out[:, e] = res.results[e]["out"].reshape(B, N, D)
    if trace:
        return out, res
    return out
